# revision 18
# baseline (speedup 1.0000x reference)
"""Trainium2 Bass kernel for nn_LlamaAttention_cam (sparse attention + CaM merge).

Sharding: tensor-parallel over heads across 8 NeuronCores (2 heads/core).
Each core computes its heads' QKV projections, RoPE, masked attention
(start+recent keep mask), CaM rank-1 correction for the last chunk, and a
partial o_proj (its 256 columns of x against the matching 256 rows of Wo^T).
The host sums the 8 partial outputs (the reduction of the head-parallel
o_proj), which replaces the all-reduce.

Measured on HW (NTFF profile, max core): 624us f32r baseline -> 175.5us.
Optimizations:
 - fp16 matmul streams; host-packed resident weights (one wide DMA each,
   Wq/Wk split per-head so the first matmul's working set is minimal)
 - K projected only for the 1023 kept keys, roped directly into a PACKED
   [D, 1024] key layout (8 attention blocks instead of 9, only the final
   pad lane masked via exp bias); V repacked into the same order with a few
   partition-offset SBUF copies
 - softmax denominator: DVE block-sum + ones[128,128] matmul (broadcast
   rows) + reciprocal_approx_fast
 - attention software-pipelined (scores/exp of combo i before AV of i-1),
   o_proj trails two slots; CaM q-block runs first so the tail is clean
"""

import sys

for _p in ("/opt/trn_rl_repo",):
    if _p not in sys.path:
        sys.path.append(_p)

import numpy as np

import concourse.bass as bass
import concourse.mybir as mybir
import concourse.tile as tile
from concourse import bacc, bass_utils

F32 = mybir.dt.float32
F16 = mybir.dt.float16
AF = mybir.ActivationFunctionType

T = 2048
DM = 2048
H = 16
D = 128
NCORES = 8
HL = H // NCORES          # heads per core = 2
JC = HL * D               # local attn width = 256
SB = 204                  # start keep
RB = 819                  # recent keep
EV = T - RB               # 1229 (first recent key; CaM source row)
KC = DM // 128            # 16 model-dim chunks
TB = T // 512             # 4 t-blocks of 512
NBP = 8                   # packed key blocks (1023 kept + 1 pad)
# packed K ranges: (orig c0, orig c1, packed p0); kept = [0,204) u [1229,2048)
PK = [(0, 204, 0), (1229, 1536, 204), (1536, 2048, 511)]
# V kept chunks per t-block (orig chunk indices)
VCHUNKS = {0: [0, 1], 2: [9, 10, 11], 3: [12, 13, 14, 15]}
# vtp repack: vtp[b][d0:d0+run] = vt[src][s0:s0+run]
# packed j < 204 -> orig j ; packed j >= 204 -> orig j + 1025
VPACK = []
for _b in range(1, NBP):
    _j = _b * 128
    _segs = []
    while _j < (_b + 1) * 128 and _j < 1023:
        _o = _j if _j < 204 else _j + 1025
        _end_keep = 204 if _j < 204 else 1023
        _chunk_end = (_o // 128 + 1) * 128
        _run = min(_end_keep - _j, _chunk_end - _o, (_b + 1) * 128 - _j)
        _segs.append((_j - _b * 128, _o // 128, _o % 128, _run))
        _j += _run
    VPACK.append((_b, _segs))

# jax.random.uniform(jax.random.key(42), (1, 16)) -- fixed by the reference
UFULL = [0.5940065383911133, 0.43801307678222656, 0.6285691261291504,
         0.007912039756774902, 0.2783470153808594, 0.7976179122924805,
         0.8521497249603271, 0.9625306129455566, 0.6765649318695068,
         0.11104440689086914, 0.49599289894104004, 0.7311437129974365,
         0.18970704078674316, 0.1544198989868164, 0.03802835941314697,
         0.3355926275253296]


def _build_nc():
    nc = bacc.Bacc("TRN2", target_bir_lowering=False, debug=False,
                   num_devices=NCORES)
    hsT = nc.dram_tensor("hsT", [DM, T], F16, kind="ExternalInput").ap()
    wqP = nc.dram_tensor("wqP", [128, HL * KC * 128], F16,
                         kind="ExternalInput").ap()
    wkP = nc.dram_tensor("wkP", [128, HL * KC * 128], F16,
                         kind="ExternalInput").ap()
    wvP = nc.dram_tensor("wvP", [128, KC * JC], F16, kind="ExternalInput").ap()
    woP = nc.dram_tensor("woP", [128, HL * DM], F16, kind="ExternalInput").ap()
    cosT = nc.dram_tensor("cosT", [D, T], F16, kind="ExternalInput").ap()
    sinTs = nc.dram_tensor("sinTs", [D, T], F16, kind="ExternalInput").ap()
    u2 = nc.dram_tensor("u2", [1, HL], F32, kind="ExternalInput").ap()
    biases = nc.dram_tensor("biases", [128, 2], F32, kind="ExternalInput").ap()
    po = nc.dram_tensor("po", [T, DM], F16, kind="ExternalOutput").ap()

    with tile.TileContext(nc) as tc:
        with (
            tc.tile_pool(name="epool", bufs=20) as pe_pool,    # f16 [128,512] E tiles
            tc.tile_pool(name="esum", bufs=2) as pesum,        # f16 [128,512]
            tc.tile_pool(name="tmp", bufs=10) as ptmp,         # f16 [128,512] transients
            tc.tile_pool(name="ostage", bufs=2) as postg,      # f16 [128,2048]
            tc.tile_pool(name="resid", bufs=1) as pres,        # long-lived
            tc.tile_pool(name="rows", bufs=8) as prow,         # small [1,*] tiles
            tc.tile_pool(name="rbf", bufs=2) as prbf,          # f32 [128,512] 1/denom
            tc.tile_pool(name="psA", bufs=3, space="PSUM") as ppsA,   # psq/psk/scores
            tc.tile_pool(name="psB", bufs=3, space="PSUM") as ppsB,   # psv/psav/pso
            tc.tile_pool(name="psD", bufs=2, space="PSUM") as ppsD,   # dn/pssr/pscr
        ):
            # ---- resident weights / tables; DMA order = need order
            # (K proj of t-block 0 runs first, then Q, V, later t-blocks) ----
            wk_sb = pres.tile([128, HL * KC * 128], F16, tag="wk")
            nc.sync.dma_start(wk_sb[:, 0:KC * 128], wkP[:, 0:KC * 128])
            hst = [pres.tile([128, T], F16, tag=f"hs{k}", name=f"hs{k}")
                   for k in range(KC)]
            for k in range(KC):
                nc.sync.dma_start(hst[k][:, 0:512],
                                  hsT[k * 128:(k + 1) * 128, 0:512])
            cosT_sb = pres.tile([D, T], F16, tag="cos")
            sinTs_sb = pres.tile([D, T], F16, tag="sin")
            nc.sync.dma_start(cosT_sb[:, 0:512], cosT[:, 0:512])
            nc.sync.dma_start(sinTs_sb[:, 0:512], sinTs[:, 0:512])
            nc.sync.dma_start(wk_sb[:, KC * 128:], wkP[:, KC * 128:])
            wq_sb = pres.tile([128, HL * KC * 128], F16, tag="wq")
            nc.sync.dma_start(wq_sb[:], wqP[:])
            wv_sb = pres.tile([128, KC * JC], F16, tag="wv")
            nc.sync.dma_start(wv_sb[:], wvP[:])
            for k in range(KC):
                nc.sync.dma_start(hst[k][:, 512:1024],
                                  hsT[k * 128:(k + 1) * 128, 512:1024])
            nc.sync.dma_start(cosT_sb[:, 512:T], cosT[:, 512:T])
            nc.sync.dma_start(sinTs_sb[:, 512:T], sinTs[:, 512:T])
            for k in range(KC):
                nc.sync.dma_start(hst[k][:, 1024:T],
                                  hsT[k * 128:(k + 1) * 128, 1024:T])
            wo_sb = pres.tile([128, HL * DM], F16, tag="wo")
            nc.sync.dma_start(wo_sb[:], woP[:])
            biases_sb = pres.tile([128, 2], F32, tag="biases")
            nc.sync.dma_start(biases_sb[:], biases[:])
            u2_sb = pres.tile([1, HL], F32, tag="u2")
            nc.sync.dma_start(u2_sb[:], u2[:])

            def wq(kc, l):
                return wq_sb[:, l * KC * 128 + kc * 128:
                             l * KC * 128 + (kc + 1) * 128]

            def wk(kc, l):
                return wk_sb[:, l * KC * 128 + kc * 128:
                             l * KC * 128 + (kc + 1) * 128]

            def wv(kc):
                return wv_sb[:, kc * JC:(kc + 1) * JC]

            def wo(l, mb):
                return wo_sb[:, l * DM + mb * 512: l * DM + (mb + 1) * 512]

            ones128 = pres.tile([128, 128], F16, tag="ones128")
            nc.vector.memset(ones128[:], 1.0)

            # rope'd q [d, t]; packed rope'd k [d, 1024]; packed v [key, d]
            qrT = [pres.tile([D, T], F16, tag=f"qrT{l}", name=f"qrT{l}")
                   for l in range(HL)]
            krTp = [pres.tile([D, 1024], F16, tag=f"krTp{l}", name=f"krTp{l}")
                    for l in range(HL)]
            vt = {ti: pres.tile([128, JC], F16, tag=f"vt{ti}", name=f"vt{ti}")
                  for ti in [c for cs in VCHUNKS.values() for c in cs]}
            vtp = [None] * NBP
            for b in range(1, NBP):
                vtp[b] = pres.tile([128, JC], F16, tag=f"vtp{b}",
                                   name=f"vtp{b}")
            outT = [pres.tile([D, T], F16, tag=f"outT{l}", name=f"outT{l}")
                    for l in range(HL)]

            for l in range(HL):
                nc.vector.memset(krTp[l][:, 1023:1024], 0.0)  # pad key
            zrow = pres.tile([1, JC], F16, tag="zrow")
            nc.vector.memset(zrow[:], 0.0)
            nc.gpsimd.dma_start(vtp[NBP - 1][127:128, :], zrow[:])  # pad V row

            def rope(ps_ap, dst_ap, c0, c1):
                # dst = ps*cos[c0:c1] + halfswap(ps)*sinTs[c0:c1]
                w = c1 - c0
                raw = ptmp.tile([128, 512], F16, tag="tmp")
                nc.scalar.copy(raw[:, 0:w], ps_ap)
                sh = ptmp.tile([128, 512], F16, tag="tmp")
                nc.gpsimd.dma_start(sh[0:64, 0:w], raw[64:128, 0:w])
                nc.gpsimd.dma_start(sh[64:128, 0:w], raw[0:64, 0:w])
                t1 = ptmp.tile([128, 512], F16, tag="tmp")
                nc.vector.tensor_mul(t1[:, 0:w], raw[:, 0:w], cosT_sb[:, c0:c1])
                t2 = ptmp.tile([128, 512], F16, tag="tmp")
                nc.vector.tensor_mul(t2[:, 0:w], sh[:, 0:w], sinTs_sb[:, c0:c1])
                nc.vector.tensor_add(dst_ap, t1[:, 0:w], t2[:, 0:w])

            # ---------------- phase 1: projections + rope ----------------
            kr_by_tb = {rng[0] // 512: rng for rng in PK}
            for tb in range(TB):
                c0, c1 = tb * 512, tb * 512 + 512
                if tb in kr_by_tb:
                    g0, g1, p0 = kr_by_tb[tb]
                    w = g1 - g0
                    for l in range(HL):
                        psk = ppsA.tile([128, 512], F32, tag="ps")
                        for kc in range(KC):
                            nc.tensor.matmul(psk[:, 0:w], wk(kc, l),
                                             hst[kc][:, g0:g1],
                                             start=(kc == 0), stop=(kc == KC - 1))
                        rope(psk[:, 0:w], krTp[l][:, p0:p0 + w], g0, g1)
                for l in range(HL):
                    psq = ppsA.tile([128, 512], F32, tag="ps")
                    for kc in range(KC):
                        nc.tensor.matmul(psq[:], wq(kc, l), hst[kc][:, c0:c1],
                                         start=(kc == 0), stop=(kc == KC - 1))
                    rope(psq[:], qrT[l][:, c0:c1], c0, c1)
                for ti in VCHUNKS.get(tb, []):
                    psv = ppsB.tile([128, JC], F32, tag="psb")
                    for kc in range(KC):
                        nc.tensor.matmul(psv[:],
                                         hst[kc][:, ti * 128:(ti + 1) * 128],
                                         wv(kc), start=(kc == 0),
                                         stop=(kc == KC - 1))
                    nc.scalar.copy(vt[ti][:], psv[:])
                    # repack this chunk's rows into packed-key V blocks
                    # as soon as they exist (keeps the copies off the
                    # attention-start critical path)
                    for b, segs in VPACK:
                        for (d0, src, s0, run) in segs:
                            if src == ti:
                                nc.gpsimd.dma_start(
                                    vtp[b][d0:d0 + run, :],
                                    vt[src][s0:s0 + run, :])

            def vblk(b):
                return vt[0] if b == 0 else vtp[b]

            # ---------------- phase 2: attention + pipelined o_proj ----------------
            def scores_exp(l, qb):
                qs = slice(qb * 512, qb * 512 + 512)
                E = []
                for b in range(NBP):
                    pst = ppsA.tile([128, 512], F32, tag="ps")
                    nc.tensor.matmul(pst[:], krTp[l][:, b * 128:(b + 1) * 128],
                                     qrT[l][:, qs], start=True, stop=True)
                    e = pe_pool.tile([128, 512], F16, tag="e")
                    bias = (biases_sb[:, 0:1] if b == NBP - 1 else 0.0)
                    nc.scalar.activation(e[:], pst[:], AF.Exp, bias=bias)
                    E.append(e)
                esum = pesum.tile([128, 512], F16, tag="esum")
                nc.vector.tensor_add(esum[:], E[0][:], E[1][:])
                for bi in range(2, NBP):
                    nc.vector.tensor_add(esum[:], esum[:], E[bi][:])
                return E, esum

            def av_norm(l, qb, E, esum):
                qs = slice(qb * 512, qb * 512 + 512)
                psav = ppsB.tile([128, 512], F32, tag="psb")
                for b in range(NBP):
                    nc.tensor.matmul(psav[:], vblk(b)[:, l * D:(l + 1) * D],
                                     E[b][:], start=(b == 0),
                                     stop=(b == NBP - 1))
                # denominator broadcast to all 128 rows, then fast recip
                psdn = ppsD.tile([128, 512], F32, tag="dn")
                nc.tensor.matmul(psdn[:], ones128[:], esum[:],
                                 start=True, stop=True)
                rbf = prbf.tile([128, 512], F32, tag="rbf")
                nc.vector.reciprocal_approx_fast(out=rbf[:], in_=psdn[:])

                if qb == TB - 1:
                    # ---- CaM: bernoulli draw + rank-1 merge on t>=1792 ----
                    # sum over recent keys (packed >= 204): mask block 1
                    # (rows < 76 are start keys), then blocks 2..7
                    e1m = ptmp.tile([128, 512], F16, tag="tmp")
                    nc.vector.tensor_scalar_mul(e1m[:, 0:256],
                                                E[1][:, 256:512],
                                                biases_sb[:, 1:2])
                    pssr = ppsD.tile([128, 256], F32, tag="dn")
                    nc.tensor.matmul(pssr[:], ones128[:], e1m[:, 0:256],
                                     start=True, stop=False)
                    for b in range(2, NBP):
                        nc.tensor.matmul(pssr[:], ones128[:],
                                         E[b][:, 256:512],
                                         start=False, stop=(b == NBP - 1))
                    # E row of evicted key 1229 = packed 204 (block 1, row 76)
                    erow = prow.tile([1, 256], F16, tag="row256")
                    nc.gpsimd.dma_start(erow[:], E[1][76:77, 256:512])
                    srec = prow.tile([1, 256], F32, tag="row256f")
                    nc.vector.tensor_sub(srec[:], pssr[0:1, :], erow[:])
                    # scalars at t = 2047 (col 255 of the 256-wide rows)
                    r_last = rbf[0:1, 511:512]
                    num = prow.tile([1, 1], F32, tag="sc")
                    nc.vector.tensor_mul(num[:], erow[0:1, 255:256], r_last)
                    mean = prow.tile([1, 1], F32, tag="sc")
                    nc.vector.tensor_mul(mean[:], srec[0:1, 255:256], r_last)
                    nc.vector.tensor_scalar_mul(mean[:], mean[:], 1.0 / 818.0)
                    nc.vector.tensor_scalar_add(mean[:], mean[:], 1e-6)
                    um = prow.tile([1, 1], F32, tag="sc")
                    nc.vector.tensor_mul(um[:], u2_sb[0:1, l:l + 1], mean[:])
                    bern = prow.tile([1, 1], F32, tag="sc")
                    nc.vector.tensor_tensor(bern[:], um[:], num[:],
                                            mybir.AluOpType.is_lt)
                    bs = prow.tile([1, 1], F32, tag="sc")
                    nc.vector.tensor_scalar_mul(bs[:], bern[:], 1.0 / RB)
                    coef = prow.tile([1, 256], F16, tag="row256")
                    nc.vector.tensor_scalar_mul(coef[:], srec[:], bs[:])
                    vrow = prow.tile([1, D], F16, tag="vrow")
                    nc.gpsimd.dma_start(vrow[:],
                                          vtp[1][76:77, l * D:(l + 1) * D])
                    pscr = ppsD.tile([128, 256], F32, tag="dn")
                    nc.tensor.matmul(pscr[:], vrow[:], coef[:],
                                     start=True, stop=True)
                # normalize columns by 1/denom
                nc.vector.tensor_mul(outT[l][:, qs], psav[:], rbf[:])
                if qb == TB - 1:
                    corr = ptmp.tile([128, 512], F16, tag="tmp")
                    nc.vector.tensor_mul(corr[:, 0:256], pscr[:],
                                         rbf[:, 256:512])
                    nc.vector.tensor_add(outT[l][:, 1792:2048],
                                         outT[l][:, 1792:2048],
                                         corr[:, 0:256])

            def o_proj(qb, fine=False):
                ost = postg.tile([128, DM], F16, tag="ostage")
                for i, ti in enumerate(range(qb * 4, qb * 4 + 4)):
                    for mb in range(TB):
                        pso = ppsB.tile([128, 512], F32, tag="psb")
                        for l in range(HL):
                            nc.tensor.matmul(
                                pso[:], outT[l][:, ti * 128:(ti + 1) * 128],
                                wo(l, mb), start=(l == 0), stop=(l == HL - 1))
                        dst = ost[:, mb * 512:(mb + 1) * 512]
                        if (i * TB + mb) % 2 == 0:
                            nc.scalar.copy(dst, pso[:])
                        else:
                            nc.vector.tensor_copy(dst, pso[:])
                        if fine and mb % 2 == 1:
                            nc.gpsimd.dma_start(
                                po[ti * 128:(ti + 1) * 128,
                                   (mb - 1) * 512:(mb + 1) * 512],
                                ost[:, (mb - 1) * 512:(mb + 1) * 512])
                    if not fine:
                        nc.sync.dma_start(po[ti * 128:(ti + 1) * 128, :],
                                          ost[:])
                    if ti != qb * 4 + 3:
                        ost = postg.tile([128, DM], F16, tag="ostage")

            qbseq = [0, 3, 1, 2]
            combos = [(l, qb) for qb in qbseq for l in range(HL)]
            pend = None
            for i, (l, qb) in enumerate(combos):
                cur = scores_exp(l, qb)
                if pend is not None:
                    av_norm(*pend)
                if i >= 3 and i % 2 == 1:
                    o_proj(qbseq[(i - 3) // 2])
                pend = (l, qb, *cur)
            av_norm(*pend)
            o_proj(qbseq[-1], fine=True)

    nc.compile()
    return nc


_NC_CACHE = None


def _get_nc():
    global _NC_CACHE
    if _NC_CACHE is None:
        _NC_CACHE = _build_nc()
    return _NC_CACHE


def _pack_w_lmajor(WT):
    # WT: [DM, JC] -> [128, HL*KC*128], head-major then kc
    out = []
    for l in range(HL):
        Wl = WT[:, l * 128:(l + 1) * 128]                  # [DM, 128]
        out.append(Wl.reshape(KC, 128, 128).transpose(1, 0, 2)
                   .reshape(128, KC * 128))
    return np.ascontiguousarray(np.concatenate(out, axis=1))


def make_in_maps(hidden_states, Wq, Wk, Wv, Wo):
    hs = np.asarray(hidden_states, np.float32).reshape(T, DM)
    hs = np.nan_to_num(hs, nan=0.0, posinf=1e4, neginf=-1e4)
    hsT = np.ascontiguousarray(hs.T.astype(np.float16))
    Wq = np.asarray(Wq, np.float32)
    Wk = np.asarray(Wk, np.float32)
    Wv = np.asarray(Wv, np.float32)
    Wo = np.asarray(Wo, np.float32)

    inv_freq = 1.0 / (10000.0 ** (np.arange(0, D, 2, dtype=np.float32) / D))
    freqs = np.arange(T, dtype=np.float32)[:, None] * inv_freq[None, :]
    emb = np.concatenate([freqs, freqs], axis=-1)          # [T, D]
    cosT = np.ascontiguousarray(np.cos(emb).T.astype(np.float16))
    sinT = np.sin(emb).T.astype(np.float32)
    sinTs = np.ascontiguousarray(
        np.concatenate([-sinT[:D // 2], sinT[D // 2:]], axis=0).astype(np.float16))

    u_full = np.asarray(UFULL, np.float32).reshape(1, H)

    bias_np = np.zeros((128, 2), np.float32)
    bias_np[127, 0] = -60000.0   # packed pad lane in block 7
    bias_np[76:, 1] = 1.0        # CaM recent-sum mask for block 1

    scale = 1.0 / np.sqrt(np.float32(D))
    in_maps = []
    for c in range(NCORES):
        js = slice(c * JC, (c + 1) * JC)
        in_maps.append({
            "hsT": hsT,
            "wqP": _pack_w_lmajor((Wq[js, :].T * scale).astype(np.float16)),
            "wkP": _pack_w_lmajor(Wk[js, :].T.astype(np.float16)),
            "wvP": np.ascontiguousarray(
                Wv[js, :].T.astype(np.float16).reshape(KC, 128, JC)
                .transpose(1, 0, 2).reshape(128, KC * JC)),
            "woP": np.ascontiguousarray(
                Wo[:, js].T.astype(np.float16).reshape(HL, 128, DM)
                .transpose(1, 0, 2).reshape(128, HL * DM)),
            "cosT": cosT,
            "sinTs": sinTs,
            "u2": np.ascontiguousarray(u_full[:, c * HL:(c + 1) * HL]),
            "biases": bias_np,
        })
    return in_maps


def kernel(hidden_states, Wq, Wk, Wv, Wo):
    nc = _get_nc()
    in_maps = make_in_maps(hidden_states, Wq, Wk, Wv, Wo)
    res = bass_utils.run_bass_kernel_spmd(nc, in_maps,
                                          core_ids=list(range(NCORES)))
    out = np.zeros((T, DM), np.float64)
    for c in range(NCORES):
        out += res.results[c]["po"].astype(np.float64)
    out = np.nan_to_num(out.astype(np.float32), nan=0.0, posinf=1e4,
                        neginf=-1e4)
    return out.reshape(1, T, DM)


# revision 19
# speedup vs baseline: 1.1072x; 1.1072x over previous
"""Trainium2 Bass kernel for nn_LlamaAttention_cam (sparse attention + CaM merge).

Sharding: tensor-parallel over heads across 8 NeuronCores (2 heads/core).
Each core computes its heads' QKV projections, RoPE, masked attention
(start+recent keep mask), CaM rank-1 correction for the last chunk, and a
partial o_proj (its 256 columns of x against the matching 256 rows of Wo^T).
The host sums the 8 partial outputs (the reduction of the head-parallel
o_proj), which replaces the all-reduce.

Measured on HW (NTFF profile, max core): 624us f32r baseline -> 175.5us.
Optimizations:
 - fp16 matmul streams; host-packed resident weights (one wide DMA each,
   Wq/Wk split per-head so the first matmul's working set is minimal)
 - K projected only for the 1023 kept keys, roped directly into a PACKED
   [D, 1024] key layout (8 attention blocks instead of 9, only the final
   pad lane masked via exp bias); V repacked into the same order with a few
   partition-offset SBUF copies
 - softmax denominator: DVE block-sum + ones[128,128] matmul (broadcast
   rows) + reciprocal_approx_fast
 - attention software-pipelined (scores/exp of combo i before AV of i-1),
   o_proj trails two slots; CaM q-block runs first so the tail is clean
"""

import sys

for _p in ("/opt/trn_rl_repo",):
    if _p not in sys.path:
        sys.path.append(_p)

import numpy as np

import concourse.bass as bass
import concourse.mybir as mybir
import concourse.tile as tile
from concourse import bacc, bass_utils

F32 = mybir.dt.float32
F16 = mybir.dt.float16
AF = mybir.ActivationFunctionType

T = 2048
DM = 2048
H = 16
D = 128
NCORES = 8
HL = H // NCORES          # heads per core = 2
JC = HL * D               # local attn width = 256
SB = 204                  # start keep
RB = 819                  # recent keep
EV = T - RB               # 1229 (first recent key; CaM source row)
KC = DM // 128            # 16 model-dim chunks
TB = T // 512             # 4 t-blocks of 512
NBP = 8                   # packed key blocks (1023 kept + 1 pad)
# packed K ranges: (orig c0, orig c1, packed p0); kept = [0,204) u [1229,2048)
PK = [(0, 204, 0), (1229, 1536, 204), (1536, 2048, 511)]
# V kept chunks per t-block (orig chunk indices)
VCHUNKS = {0: [0, 1], 2: [9, 10, 11], 3: [12, 13, 14, 15]}
# vtp repack: vtp[b][d0:d0+run] = vt[src][s0:s0+run]
# packed j < 204 -> orig j ; packed j >= 204 -> orig j + 1025
VPACK = []
for _b in range(1, NBP):
    _j = _b * 128
    _segs = []
    while _j < (_b + 1) * 128 and _j < 1023:
        _o = _j if _j < 204 else _j + 1025
        _end_keep = 204 if _j < 204 else 1023
        _chunk_end = (_o // 128 + 1) * 128
        _run = min(_end_keep - _j, _chunk_end - _o, (_b + 1) * 128 - _j)
        _segs.append((_j - _b * 128, _o // 128, _o % 128, _run))
        _j += _run
    VPACK.append((_b, _segs))

# jax.random.uniform(jax.random.key(42), (1, 16)) -- fixed by the reference
UFULL = [0.5940065383911133, 0.43801307678222656, 0.6285691261291504,
         0.007912039756774902, 0.2783470153808594, 0.7976179122924805,
         0.8521497249603271, 0.9625306129455566, 0.6765649318695068,
         0.11104440689086914, 0.49599289894104004, 0.7311437129974365,
         0.18970704078674316, 0.1544198989868164, 0.03802835941314697,
         0.3355926275253296]


def _build_nc():
    nc = bacc.Bacc("TRN2", target_bir_lowering=False, debug=False,
                   num_devices=NCORES)
    hsT = nc.dram_tensor("hsT", [DM, T], F16, kind="ExternalInput").ap()
    wqP = nc.dram_tensor("wqP", [128, HL * KC * 128], F16,
                         kind="ExternalInput").ap()
    wkP = nc.dram_tensor("wkP", [128, HL * KC * 128], F16,
                         kind="ExternalInput").ap()
    wvP = nc.dram_tensor("wvP", [128, KC * JC], F16, kind="ExternalInput").ap()
    woP = nc.dram_tensor("woP", [128, HL * DM], F16, kind="ExternalInput").ap()
    cosT = nc.dram_tensor("cosT", [D, T], F16, kind="ExternalInput").ap()
    sinTs = nc.dram_tensor("sinTs", [D, T], F16, kind="ExternalInput").ap()
    u2 = nc.dram_tensor("u2", [1, HL], F32, kind="ExternalInput").ap()
    biases = nc.dram_tensor("biases", [128, 2], F32, kind="ExternalInput").ap()
    po = nc.dram_tensor("po", [T, DM], F16, kind="ExternalOutput").ap()

    with tile.TileContext(nc) as tc:
        with (
            tc.tile_pool(name="epool", bufs=12) as pe_pool,    # f16 [128,1024] E tiles
            tc.tile_pool(name="esum", bufs=2) as pesum,        # f16 [128,1024]
            tc.tile_pool(name="tmp", bufs=10) as ptmp,         # f16 [128,512] transients
            tc.tile_pool(name="ostage", bufs=2) as postg,      # f16 [128,2048]
            tc.tile_pool(name="resid", bufs=1) as pres,        # long-lived
            tc.tile_pool(name="rows", bufs=8) as prow,         # small [1,*] tiles
            tc.tile_pool(name="rbf", bufs=2) as prbf,          # f32 [128,512] 1/denom
            tc.tile_pool(name="psA", bufs=2, space="PSUM") as ppsA,   # psq/psk/scores
            tc.tile_pool(name="psB", bufs=3, space="PSUM") as ppsB,   # psv/psav/pso
            tc.tile_pool(name="psD", bufs=1, space="PSUM") as ppsD,   # dn/pssr/pscr
        ):
            # ---- resident weights / tables; DMA order = need order
            # (K proj of t-block 0 runs first, then Q, V, later t-blocks) ----
            wk_sb = pres.tile([128, HL * KC * 128], F16, tag="wk")
            nc.sync.dma_start(wk_sb[:, 0:KC * 128], wkP[:, 0:KC * 128])
            hst = [pres.tile([128, T], F16, tag=f"hs{k}", name=f"hs{k}")
                   for k in range(KC)]
            for k in range(KC):
                nc.sync.dma_start(hst[k][:, 0:512],
                                  hsT[k * 128:(k + 1) * 128, 0:512])
            cosT_sb = pres.tile([D, T], F16, tag="cos")
            sinTs_sb = pres.tile([D, T], F16, tag="sin")
            nc.sync.dma_start(cosT_sb[:, 0:512], cosT[:, 0:512])
            nc.sync.dma_start(sinTs_sb[:, 0:512], sinTs[:, 0:512])
            nc.sync.dma_start(wk_sb[:, KC * 128:], wkP[:, KC * 128:])
            wq_sb = pres.tile([128, HL * KC * 128], F16, tag="wq")
            nc.sync.dma_start(wq_sb[:], wqP[:])
            wv_sb = pres.tile([128, KC * JC], F16, tag="wv")
            nc.sync.dma_start(wv_sb[:], wvP[:])
            for k in range(KC):
                nc.sync.dma_start(hst[k][:, 512:1024],
                                  hsT[k * 128:(k + 1) * 128, 512:1024])
            nc.sync.dma_start(cosT_sb[:, 512:T], cosT[:, 512:T])
            nc.sync.dma_start(sinTs_sb[:, 512:T], sinTs[:, 512:T])
            for k in range(KC):
                nc.sync.dma_start(hst[k][:, 1024:T],
                                  hsT[k * 128:(k + 1) * 128, 1024:T])
            wo_sb = pres.tile([128, HL * DM], F16, tag="wo")
            nc.sync.dma_start(wo_sb[:], woP[:])
            biases_sb = pres.tile([128, 2], F32, tag="biases")
            nc.sync.dma_start(biases_sb[:], biases[:])
            u2_sb = pres.tile([1, HL], F32, tag="u2")
            nc.sync.dma_start(u2_sb[:], u2[:])

            def wq(kc, l):
                return wq_sb[:, l * KC * 128 + kc * 128:
                             l * KC * 128 + (kc + 1) * 128]

            def wk(kc, l):
                return wk_sb[:, l * KC * 128 + kc * 128:
                             l * KC * 128 + (kc + 1) * 128]

            def wv(kc):
                return wv_sb[:, kc * JC:(kc + 1) * JC]

            def wo(l, mb):
                return wo_sb[:, l * DM + mb * 512: l * DM + (mb + 1) * 512]

            ones128 = pres.tile([128, 128], F16, tag="ones128")
            nc.vector.memset(ones128[:], 1.0)

            # rope'd q [d, t]; packed rope'd k [d, 1024]; packed v [key, d]
            qrT = [pres.tile([D, T], F16, tag=f"qrT{l}", name=f"qrT{l}")
                   for l in range(HL)]
            krTp = [pres.tile([D, 1024], F16, tag=f"krTp{l}", name=f"krTp{l}")
                    for l in range(HL)]
            vt = {ti: pres.tile([128, JC], F16, tag=f"vt{ti}", name=f"vt{ti}")
                  for ti in [c for cs in VCHUNKS.values() for c in cs]}
            vtp = [None] * NBP
            for b in range(1, NBP):
                vtp[b] = pres.tile([128, JC], F16, tag=f"vtp{b}",
                                   name=f"vtp{b}")
            outT = [pres.tile([D, T], F16, tag=f"outT{l}", name=f"outT{l}")
                    for l in range(HL)]

            for l in range(HL):
                nc.vector.memset(krTp[l][:, 1023:1024], 0.0)  # pad key
            zrow = pres.tile([1, JC], F16, tag="zrow")
            nc.vector.memset(zrow[:], 0.0)
            nc.gpsimd.dma_start(vtp[NBP - 1][127:128, :], zrow[:])  # pad V row

            def rope(ps_ap, dst_ap, c0, c1):
                # dst = ps*cos[c0:c1] + halfswap(ps)*sinTs[c0:c1]
                w = c1 - c0
                raw = ptmp.tile([128, 512], F16, tag="tmp")
                nc.scalar.copy(raw[:, 0:w], ps_ap)
                sh = ptmp.tile([128, 512], F16, tag="tmp")
                nc.gpsimd.dma_start(sh[0:64, 0:w], raw[64:128, 0:w])
                nc.gpsimd.dma_start(sh[64:128, 0:w], raw[0:64, 0:w])
                t1 = ptmp.tile([128, 512], F16, tag="tmp")
                nc.vector.tensor_mul(t1[:, 0:w], raw[:, 0:w], cosT_sb[:, c0:c1])
                t2 = ptmp.tile([128, 512], F16, tag="tmp")
                nc.vector.tensor_mul(t2[:, 0:w], sh[:, 0:w], sinTs_sb[:, c0:c1])
                nc.vector.tensor_add(dst_ap, t1[:, 0:w], t2[:, 0:w])

            # ---------------- phase 1: projections + rope ----------------
            kr_by_tb = {rng[0] // 512: rng for rng in PK}
            for tb in range(TB):
                c0, c1 = tb * 512, tb * 512 + 512
                if tb in kr_by_tb:
                    g0, g1, p0 = kr_by_tb[tb]
                    w = g1 - g0
                    for l in range(HL):
                        psk = ppsA.tile([128, 512], F32, tag="ps")
                        for kc in range(KC):
                            nc.tensor.matmul(psk[:, 0:w], wk(kc, l),
                                             hst[kc][:, g0:g1],
                                             start=(kc == 0), stop=(kc == KC - 1))
                        rope(psk[:, 0:w], krTp[l][:, p0:p0 + w], g0, g1)
                for l in range(HL):
                    psq = ppsA.tile([128, 512], F32, tag="ps")
                    for kc in range(KC):
                        nc.tensor.matmul(psq[:], wq(kc, l), hst[kc][:, c0:c1],
                                         start=(kc == 0), stop=(kc == KC - 1))
                    rope(psq[:], qrT[l][:, c0:c1], c0, c1)
                for ti in VCHUNKS.get(tb, []):
                    psv = ppsB.tile([128, JC], F32, tag="psb")
                    for kc in range(KC):
                        nc.tensor.matmul(psv[:],
                                         hst[kc][:, ti * 128:(ti + 1) * 128],
                                         wv(kc), start=(kc == 0),
                                         stop=(kc == KC - 1))
                    nc.scalar.copy(vt[ti][:], psv[:])
                    # repack this chunk's rows into packed-key V blocks
                    # as soon as they exist (keeps the copies off the
                    # attention-start critical path)
                    for b, segs in VPACK:
                        for (d0, src, s0, run) in segs:
                            if src == ti:
                                nc.gpsimd.dma_start(
                                    vtp[b][d0:d0 + run, :],
                                    vt[src][s0:s0 + run, :])

            def vblk(b):
                return vt[0] if b == 0 else vtp[b]

            # ---------------- phase 2: attention + pipelined o_proj ----------------
            def scores_exp(l, pair):
                # two q-blocks' scores land in halves of one [128,1024] psum;
                # one wide exp halves the ACTIVATE count (fixed psum-access
                # overhead per instruction)
                qlo = slice(pair[0] * 512, pair[0] * 512 + 512)
                qhi = slice(pair[1] * 512, pair[1] * 512 + 512)
                E = []
                for b in range(NBP):
                    ks = krTp[l][:, b * 128:(b + 1) * 128]
                    pst = ppsA.tile([128, 1024], F32, tag="ps")
                    nc.tensor.matmul(pst[:, 0:512], ks, qrT[l][:, qlo],
                                     start=True, stop=True)
                    nc.tensor.matmul(pst[:, 512:1024], ks, qrT[l][:, qhi],
                                     start=True, stop=True)
                    e = pe_pool.tile([128, 1024], F16, tag="e")
                    bias = (biases_sb[:, 0:1] if b == NBP - 1 else 0.0)
                    nc.scalar.activation(e[:], pst[:], AF.Exp, bias=bias)
                    E.append(e)
                esum = pesum.tile([128, 1024], F16, tag="esum")
                nc.vector.tensor_add(esum[:], E[0][:], E[1][:])
                for bi in range(2, NBP):
                    nc.vector.tensor_add(esum[:], esum[:], E[bi][:])
                return E, esum

            def av_norm(l, qb, E, esum, qo):
                qs = slice(qb * 512, qb * 512 + 512)
                eh = slice(qo, qo + 512)
                psav = ppsB.tile([128, 512], F32, tag="psb")
                for b in range(NBP):
                    nc.tensor.matmul(psav[:], vblk(b)[:, l * D:(l + 1) * D],
                                     E[b][:, eh], start=(b == 0),
                                     stop=(b == NBP - 1))
                # denominator broadcast to all 128 rows, then fast recip
                psdn = ppsD.tile([128, 512], F32, tag="dn")
                nc.tensor.matmul(psdn[:], ones128[:], esum[:, eh],
                                 start=True, stop=True)
                rbf = prbf.tile([128, 512], F32, tag="rbf")
                nc.vector.reciprocal_approx_fast(out=rbf[:], in_=psdn[:])

                if qb == TB - 1:
                    # ---- CaM: bernoulli draw + rank-1 merge on t>=1792 ----
                    # sum over recent keys (packed >= 204): mask block 1
                    # (rows < 76 are start keys), then blocks 2..7
                    cs = slice(qo + 256, qo + 512)
                    e1m = ptmp.tile([128, 512], F16, tag="tmp")
                    nc.vector.tensor_scalar_mul(e1m[:, 0:256],
                                                E[1][:, cs],
                                                biases_sb[:, 1:2])
                    pssr = ppsD.tile([128, 256], F32, tag="dn")
                    nc.tensor.matmul(pssr[:], ones128[:], e1m[:, 0:256],
                                     start=True, stop=False)
                    for b in range(2, NBP):
                        nc.tensor.matmul(pssr[:], ones128[:],
                                         E[b][:, cs],
                                         start=False, stop=(b == NBP - 1))
                    # E row of evicted key 1229 = packed 204 (block 1, row 76)
                    erow = prow.tile([1, 256], F16, tag="row256")
                    nc.gpsimd.dma_start(erow[:], E[1][76:77, cs])
                    srec = prow.tile([1, 256], F32, tag="row256f")
                    nc.vector.tensor_sub(srec[:], pssr[0:1, :], erow[:])
                    # scalars at t = 2047 (col 255 of the 256-wide rows)
                    r_last = rbf[0:1, 511:512]
                    num = prow.tile([1, 1], F32, tag="sc")
                    nc.vector.tensor_mul(num[:], erow[0:1, 255:256], r_last)
                    mean = prow.tile([1, 1], F32, tag="sc")
                    nc.vector.tensor_mul(mean[:], srec[0:1, 255:256], r_last)
                    nc.vector.tensor_scalar_mul(mean[:], mean[:], 1.0 / 818.0)
                    nc.vector.tensor_scalar_add(mean[:], mean[:], 1e-6)
                    um = prow.tile([1, 1], F32, tag="sc")
                    nc.vector.tensor_mul(um[:], u2_sb[0:1, l:l + 1], mean[:])
                    bern = prow.tile([1, 1], F32, tag="sc")
                    nc.vector.tensor_tensor(bern[:], um[:], num[:],
                                            mybir.AluOpType.is_lt)
                    bs = prow.tile([1, 1], F32, tag="sc")
                    nc.vector.tensor_scalar_mul(bs[:], bern[:], 1.0 / RB)
                    coef = prow.tile([1, 256], F16, tag="row256")
                    nc.vector.tensor_scalar_mul(coef[:], srec[:], bs[:])
                    vrow = prow.tile([1, D], F16, tag="vrow")
                    nc.gpsimd.dma_start(vrow[:],
                                          vtp[1][76:77, l * D:(l + 1) * D])
                    pscr = ppsD.tile([128, 256], F32, tag="dn")
                    nc.tensor.matmul(pscr[:], vrow[:], coef[:],
                                     start=True, stop=True)
                # normalize columns by 1/denom
                nc.vector.tensor_mul(outT[l][:, qs], psav[:], rbf[:])
                if qb == TB - 1:
                    corr = ptmp.tile([128, 512], F16, tag="tmp")
                    nc.vector.tensor_mul(corr[:, 0:256], pscr[:],
                                         rbf[:, 256:512])
                    nc.vector.tensor_add(outT[l][:, 1792:2048],
                                         outT[l][:, 1792:2048],
                                         corr[:, 0:256])

            def o_proj(qb, fine=False):
                ost = postg.tile([128, DM], F16, tag="ostage")
                for i, ti in enumerate(range(qb * 4, qb * 4 + 4)):
                    for mb in range(TB):
                        pso = ppsB.tile([128, 512], F32, tag="psb")
                        for l in range(HL):
                            nc.tensor.matmul(
                                pso[:], outT[l][:, ti * 128:(ti + 1) * 128],
                                wo(l, mb), start=(l == 0), stop=(l == HL - 1))
                        dst = ost[:, mb * 512:(mb + 1) * 512]
                        if (i * TB + mb) % 2 == 0:
                            nc.scalar.copy(dst, pso[:])
                        else:
                            nc.vector.tensor_copy(dst, pso[:])
                        if fine and mb % 2 == 1:
                            nc.gpsimd.dma_start(
                                po[ti * 128:(ti + 1) * 128,
                                   (mb - 1) * 512:(mb + 1) * 512],
                                ost[:, (mb - 1) * 512:(mb + 1) * 512])
                    if not fine:
                        nc.sync.dma_start(po[ti * 128:(ti + 1) * 128, :],
                                          ost[:])
                    if ti != qb * 4 + 3:
                        ost = postg.tile([128, DM], F16, tag="ostage")

            pairs = [(2, 3), (0, 1)]
            combos = [(l, P) for P in pairs for l in range(HL)]
            pend = None
            for i, (l, P) in enumerate(combos):
                cur = scores_exp(l, P)
                if pend is not None:
                    pl, pP, pE, pes = pend
                    av_norm(pl, pP[0], pE, pes, 0)
                    av_norm(pl, pP[1], pE, pes, 512)
                if i == 3:
                    o_proj(pairs[0][0])
                    o_proj(pairs[0][1])
                pend = (l, P, *cur)
            pl, pP, pE, pes = pend
            av_norm(pl, pP[0], pE, pes, 0)
            av_norm(pl, pP[1], pE, pes, 512)
            o_proj(pairs[1][0])
            o_proj(pairs[1][1], fine=True)

    nc.compile()
    return nc


_NC_CACHE = None


def _get_nc():
    global _NC_CACHE
    if _NC_CACHE is None:
        _NC_CACHE = _build_nc()
    return _NC_CACHE


def _pack_w_lmajor(WT):
    # WT: [DM, JC] -> [128, HL*KC*128], head-major then kc
    out = []
    for l in range(HL):
        Wl = WT[:, l * 128:(l + 1) * 128]                  # [DM, 128]
        out.append(Wl.reshape(KC, 128, 128).transpose(1, 0, 2)
                   .reshape(128, KC * 128))
    return np.ascontiguousarray(np.concatenate(out, axis=1))


def make_in_maps(hidden_states, Wq, Wk, Wv, Wo):
    hs = np.asarray(hidden_states, np.float32).reshape(T, DM)
    hs = np.nan_to_num(hs, nan=0.0, posinf=1e4, neginf=-1e4)
    hsT = np.ascontiguousarray(hs.T.astype(np.float16))
    Wq = np.asarray(Wq, np.float32)
    Wk = np.asarray(Wk, np.float32)
    Wv = np.asarray(Wv, np.float32)
    Wo = np.asarray(Wo, np.float32)

    inv_freq = 1.0 / (10000.0 ** (np.arange(0, D, 2, dtype=np.float32) / D))
    freqs = np.arange(T, dtype=np.float32)[:, None] * inv_freq[None, :]
    emb = np.concatenate([freqs, freqs], axis=-1)          # [T, D]
    cosT = np.ascontiguousarray(np.cos(emb).T.astype(np.float16))
    sinT = np.sin(emb).T.astype(np.float32)
    sinTs = np.ascontiguousarray(
        np.concatenate([-sinT[:D // 2], sinT[D // 2:]], axis=0).astype(np.float16))

    u_full = np.asarray(UFULL, np.float32).reshape(1, H)

    bias_np = np.zeros((128, 2), np.float32)
    bias_np[127, 0] = -60000.0   # packed pad lane in block 7
    bias_np[76:, 1] = 1.0        # CaM recent-sum mask for block 1

    scale = 1.0 / np.sqrt(np.float32(D))
    in_maps = []
    for c in range(NCORES):
        js = slice(c * JC, (c + 1) * JC)
        in_maps.append({
            "hsT": hsT,
            "wqP": _pack_w_lmajor((Wq[js, :].T * scale).astype(np.float16)),
            "wkP": _pack_w_lmajor(Wk[js, :].T.astype(np.float16)),
            "wvP": np.ascontiguousarray(
                Wv[js, :].T.astype(np.float16).reshape(KC, 128, JC)
                .transpose(1, 0, 2).reshape(128, KC * JC)),
            "woP": np.ascontiguousarray(
                Wo[:, js].T.astype(np.float16).reshape(HL, 128, DM)
                .transpose(1, 0, 2).reshape(128, HL * DM)),
            "cosT": cosT,
            "sinTs": sinTs,
            "u2": np.ascontiguousarray(u_full[:, c * HL:(c + 1) * HL]),
            "biases": bias_np,
        })
    return in_maps


def kernel(hidden_states, Wq, Wk, Wv, Wo):
    nc = _get_nc()
    in_maps = make_in_maps(hidden_states, Wq, Wk, Wv, Wo)
    res = bass_utils.run_bass_kernel_spmd(nc, in_maps,
                                          core_ids=list(range(NCORES)))
    out = np.zeros((T, DM), np.float64)
    for c in range(NCORES):
        out += res.results[c]["po"].astype(np.float64)
    out = np.nan_to_num(out.astype(np.float32), nan=0.0, posinf=1e4,
                        neginf=-1e4)
    return out.reshape(1, T, DM)


# revision 21
# speedup vs baseline: 1.1271x; 1.0179x over previous
"""Trainium2 Bass kernel for nn_LlamaAttention_cam (sparse attention + CaM merge).

Sharding: tensor-parallel over heads across 8 NeuronCores (2 heads/core).
Each core computes its heads' QKV projections, RoPE, masked attention
(start+recent keep mask), CaM rank-1 correction for the last chunk, and a
partial o_proj (its 256 columns of x against the matching 256 rows of Wo^T).
The host sums the 8 partial outputs (the reduction of the head-parallel
o_proj), which replaces the all-reduce.

Measured on HW (NTFF profile, max core): 624us f32r baseline -> ~176-182us
(device has ~±13% run-to-run variance). Optimizations:
 - fp16 matmul streams; host-packed resident weights (one wide DMA each,
   Wq/Wk split per-head so the first matmul's working set is minimal)
 - K projected only for the 1023 kept keys, roped directly into a PACKED
   [D, 1024] key layout (8 attention blocks instead of 9, only the final
   pad lane masked via exp bias); V repacked into the same order with a few
   partition-offset SBUF copies
 - softmax denominator: DVE block-sum + ones[128,128] matmul (broadcast
   rows) + reciprocal_approx_fast
 - attention software-pipelined (scores/exp of combo i before AV of i-1),
   o_proj trails a slot behind; CaM q-blocks run first so the tail is clean
 - q-blocks paired: each exp covers [128,1024] (two q-blocks' scores in one
   psum tile), halving the scalar engine's per-instruction psum-access cost
"""

import sys

for _p in ("/opt/trn_rl_repo",):
    if _p not in sys.path:
        sys.path.append(_p)

import numpy as np

import concourse.bass as bass
import concourse.mybir as mybir
import concourse.tile as tile
from concourse import bacc, bass_utils

F32 = mybir.dt.float32
F16 = mybir.dt.float16
AF = mybir.ActivationFunctionType

T = 2048
DM = 2048
H = 16
D = 128
NCORES = 8
HL = H // NCORES          # heads per core = 2
JC = HL * D               # local attn width = 256
SB = 204                  # start keep
RB = 819                  # recent keep
EV = T - RB               # 1229 (first recent key; CaM source row)
KC = DM // 128            # 16 model-dim chunks
TB = T // 512             # 4 t-blocks of 512
NBP = 8                   # packed key blocks (1023 kept + 1 pad)
# packed K ranges: (orig c0, orig c1, packed p0); kept = [0,204) u [1229,2048)
PK = [(0, 204, 0), (1229, 1536, 204), (1536, 2048, 511)]
# V kept chunks per t-block (orig chunk indices)
VCHUNKS = {0: [0, 1], 2: [9, 10, 11], 3: [12, 13, 14, 15]}
# vtp repack: vtp[b][d0:d0+run] = vt[src][s0:s0+run]
# packed j < 204 -> orig j ; packed j >= 204 -> orig j + 1025
VPACK = []
for _b in range(1, NBP):
    _j = _b * 128
    _segs = []
    while _j < (_b + 1) * 128 and _j < 1023:
        _o = _j if _j < 204 else _j + 1025
        _end_keep = 204 if _j < 204 else 1023
        _chunk_end = (_o // 128 + 1) * 128
        _run = min(_end_keep - _j, _chunk_end - _o, (_b + 1) * 128 - _j)
        _segs.append((_j - _b * 128, _o // 128, _o % 128, _run))
        _j += _run
    VPACK.append((_b, _segs))

# jax.random.uniform(jax.random.key(42), (1, 16)) -- fixed by the reference
UFULL = [0.5940065383911133, 0.43801307678222656, 0.6285691261291504,
         0.007912039756774902, 0.2783470153808594, 0.7976179122924805,
         0.8521497249603271, 0.9625306129455566, 0.6765649318695068,
         0.11104440689086914, 0.49599289894104004, 0.7311437129974365,
         0.18970704078674316, 0.1544198989868164, 0.03802835941314697,
         0.3355926275253296]


def _build_nc():
    nc = bacc.Bacc("TRN2", target_bir_lowering=False, debug=False,
                   num_devices=NCORES)
    hsT = nc.dram_tensor("hsT", [DM, T], F16, kind="ExternalInput").ap()
    wqP = nc.dram_tensor("wqP", [128, HL * KC * 128], F16,
                         kind="ExternalInput").ap()
    wkP = nc.dram_tensor("wkP", [128, HL * KC * 128], F16,
                         kind="ExternalInput").ap()
    wvP = nc.dram_tensor("wvP", [128, KC * JC], F16, kind="ExternalInput").ap()
    woP = nc.dram_tensor("woP", [128, HL * DM], F16, kind="ExternalInput").ap()
    cosT = nc.dram_tensor("cosT", [D, T], F16, kind="ExternalInput").ap()
    sinTs = nc.dram_tensor("sinTs", [D, T], F16, kind="ExternalInput").ap()
    u2 = nc.dram_tensor("u2", [1, HL], F32, kind="ExternalInput").ap()
    biases = nc.dram_tensor("biases", [128, 2], F32, kind="ExternalInput").ap()
    po = nc.dram_tensor("po", [T, DM], F16, kind="ExternalOutput").ap()

    with tile.TileContext(nc) as tc:
        with (
            tc.tile_pool(name="epool", bufs=16) as pe_pool,    # f16 [128,1024] E tiles
            tc.tile_pool(name="esum", bufs=2) as pesum,        # f16 [128,1024]
            tc.tile_pool(name="tmp", bufs=10) as ptmp,         # f16 [128,512] transients
            tc.tile_pool(name="ostage", bufs=2) as postg,      # f16 [128,2048]
            tc.tile_pool(name="resid", bufs=1) as pres,        # long-lived
            tc.tile_pool(name="rows", bufs=8) as prow,         # small [1,*] tiles
            tc.tile_pool(name="rbf", bufs=2) as prbf,          # f32 [128,512] 1/denom
            tc.tile_pool(name="psA", bufs=2, space="PSUM") as ppsA,   # psq/psk/scores
            tc.tile_pool(name="psB", bufs=3, space="PSUM") as ppsB,   # psv/psav/pso
            tc.tile_pool(name="psD", bufs=1, space="PSUM") as ppsD,   # dn/pssr/pscr
        ):
            # ---- resident weights / tables; DMA order = need order
            # (K proj of t-block 0 runs first, then Q, V, later t-blocks) ----
            wk_sb = pres.tile([128, HL * KC * 128], F16, tag="wk")
            nc.sync.dma_start(wk_sb[:, 0:KC * 128], wkP[:, 0:KC * 128])
            hst = [pres.tile([128, T], F16, tag=f"hs{k}", name=f"hs{k}")
                   for k in range(KC)]
            for k in range(KC):
                nc.sync.dma_start(hst[k][:, 0:512],
                                  hsT[k * 128:(k + 1) * 128, 0:512])
            cosT_sb = pres.tile([D, T], F16, tag="cos")
            sinTs_sb = pres.tile([D, T], F16, tag="sin")
            nc.sync.dma_start(cosT_sb[:, 0:512], cosT[:, 0:512])
            nc.sync.dma_start(sinTs_sb[:, 0:512], sinTs[:, 0:512])
            nc.sync.dma_start(wk_sb[:, KC * 128:], wkP[:, KC * 128:])
            wq_sb = pres.tile([128, HL * KC * 128], F16, tag="wq")
            nc.sync.dma_start(wq_sb[:], wqP[:])
            wv_sb = pres.tile([128, KC * JC], F16, tag="wv")
            nc.sync.dma_start(wv_sb[:], wvP[:])
            for k in range(KC):
                nc.sync.dma_start(hst[k][:, 512:1024],
                                  hsT[k * 128:(k + 1) * 128, 512:1024])
            nc.sync.dma_start(cosT_sb[:, 512:T], cosT[:, 512:T])
            nc.sync.dma_start(sinTs_sb[:, 512:T], sinTs[:, 512:T])
            for k in range(KC):
                nc.sync.dma_start(hst[k][:, 1024:T],
                                  hsT[k * 128:(k + 1) * 128, 1024:T])
            wo_sb = pres.tile([128, HL * DM], F16, tag="wo")
            nc.sync.dma_start(wo_sb[:], woP[:])
            biases_sb = pres.tile([128, 2], F32, tag="biases")
            nc.sync.dma_start(biases_sb[:], biases[:])
            u2_sb = pres.tile([1, HL], F32, tag="u2")
            nc.sync.dma_start(u2_sb[:], u2[:])

            def wq(kc, l):
                return wq_sb[:, l * KC * 128 + kc * 128:
                             l * KC * 128 + (kc + 1) * 128]

            def wk(kc, l):
                return wk_sb[:, l * KC * 128 + kc * 128:
                             l * KC * 128 + (kc + 1) * 128]

            def wv(kc):
                return wv_sb[:, kc * JC:(kc + 1) * JC]

            def wo(l, mb):
                return wo_sb[:, l * DM + mb * 512: l * DM + (mb + 1) * 512]

            ones128 = pres.tile([128, 128], F16, tag="ones128")
            nc.vector.memset(ones128[:], 1.0)

            # rope'd q [d, t]; packed rope'd k [d, 1024]; packed v [key, d]
            qrT = [pres.tile([D, T], F16, tag=f"qrT{l}", name=f"qrT{l}")
                   for l in range(HL)]
            krTp = [pres.tile([D, 1024], F16, tag=f"krTp{l}", name=f"krTp{l}")
                    for l in range(HL)]
            vt = {ti: pres.tile([128, JC], F16, tag=f"vt{ti}", name=f"vt{ti}")
                  for ti in [c for cs in VCHUNKS.values() for c in cs]}
            vtp = [None] * NBP
            for b in range(1, NBP):
                vtp[b] = pres.tile([128, JC], F16, tag=f"vtp{b}",
                                   name=f"vtp{b}")
            outT = [pres.tile([D, T], F16, tag=f"outT{l}", name=f"outT{l}")
                    for l in range(HL)]

            for l in range(HL):
                nc.vector.memset(krTp[l][:, 1023:1024], 0.0)  # pad key
            zrow = pres.tile([1, JC], F16, tag="zrow")
            nc.vector.memset(zrow[:], 0.0)
            nc.gpsimd.dma_start(vtp[NBP - 1][127:128, :], zrow[:])  # pad V row

            def rope(ps_ap, dst_ap, c0, c1):
                # dst = ps*cos[c0:c1] + halfswap(ps)*sinTs[c0:c1]
                w = c1 - c0
                raw = ptmp.tile([128, 512], F16, tag="tmp")
                nc.scalar.copy(raw[:, 0:w], ps_ap)
                sh = ptmp.tile([128, 512], F16, tag="tmp")
                nc.gpsimd.dma_start(sh[0:64, 0:w], raw[64:128, 0:w])
                nc.gpsimd.dma_start(sh[64:128, 0:w], raw[0:64, 0:w])
                t1 = ptmp.tile([128, 512], F16, tag="tmp")
                nc.vector.tensor_mul(t1[:, 0:w], raw[:, 0:w], cosT_sb[:, c0:c1])
                t2 = ptmp.tile([128, 512], F16, tag="tmp")
                nc.vector.tensor_mul(t2[:, 0:w], sh[:, 0:w], sinTs_sb[:, c0:c1])
                nc.vector.tensor_add(dst_ap, t1[:, 0:w], t2[:, 0:w])

            # ---------------- phase 1: projections + rope ----------------
            kr_by_tb = {rng[0] // 512: rng for rng in PK}
            for tb in range(TB):
                c0, c1 = tb * 512, tb * 512 + 512
                if tb in kr_by_tb:
                    g0, g1, p0 = kr_by_tb[tb]
                    w = g1 - g0
                    for l in range(HL):
                        psk = ppsA.tile([128, 512], F32, tag="ps")
                        for kc in range(KC):
                            nc.tensor.matmul(psk[:, 0:w], wk(kc, l),
                                             hst[kc][:, g0:g1],
                                             start=(kc == 0), stop=(kc == KC - 1))
                        rope(psk[:, 0:w], krTp[l][:, p0:p0 + w], g0, g1)
                for l in range(HL):
                    psq = ppsA.tile([128, 512], F32, tag="ps")
                    for kc in range(KC):
                        nc.tensor.matmul(psq[:], wq(kc, l), hst[kc][:, c0:c1],
                                         start=(kc == 0), stop=(kc == KC - 1))
                    rope(psq[:], qrT[l][:, c0:c1], c0, c1)
                for ti in VCHUNKS.get(tb, []):
                    psv = ppsB.tile([128, JC], F32, tag="psb")
                    for kc in range(KC):
                        nc.tensor.matmul(psv[:],
                                         hst[kc][:, ti * 128:(ti + 1) * 128],
                                         wv(kc), start=(kc == 0),
                                         stop=(kc == KC - 1))
                    nc.scalar.copy(vt[ti][:], psv[:])
                    # repack this chunk's rows into packed-key V blocks
                    # as soon as they exist (keeps the copies off the
                    # attention-start critical path)
                    for b, segs in VPACK:
                        for (d0, src, s0, run) in segs:
                            if src == ti:
                                nc.gpsimd.dma_start(
                                    vtp[b][d0:d0 + run, :],
                                    vt[src][s0:s0 + run, :])

            def vblk(b):
                return vt[0] if b == 0 else vtp[b]

            # ---------------- phase 2: attention + pipelined o_proj ----------------
            def scores_exp(l, pair):
                # two q-blocks' scores land in halves of one [128,1024] psum;
                # one wide exp halves the ACTIVATE count (fixed psum-access
                # overhead per instruction)
                qlo = slice(pair[0] * 512, pair[0] * 512 + 512)
                qhi = slice(pair[1] * 512, pair[1] * 512 + 512)
                E = []
                for b in range(NBP):
                    ks = krTp[l][:, b * 128:(b + 1) * 128]
                    pst = ppsA.tile([128, 1024], F32, tag="ps")
                    nc.tensor.matmul(pst[:, 0:512], ks, qrT[l][:, qlo],
                                     start=True, stop=True)
                    nc.tensor.matmul(pst[:, 512:1024], ks, qrT[l][:, qhi],
                                     start=True, stop=True)
                    e = pe_pool.tile([128, 1024], F16, tag="e")
                    bias = (biases_sb[:, 0:1] if b == NBP - 1 else 0.0)
                    nc.scalar.activation(e[:], pst[:], AF.Exp, bias=bias)
                    E.append(e)
                esum = pesum.tile([128, 1024], F16, tag="esum")
                nc.vector.tensor_add(esum[:], E[0][:], E[1][:])
                for bi in range(2, NBP):
                    nc.vector.tensor_add(esum[:], esum[:], E[bi][:])
                return E, esum

            def av_norm(l, qb, E, esum, qo):
                qs = slice(qb * 512, qb * 512 + 512)
                eh = slice(qo, qo + 512)
                psav = ppsB.tile([128, 512], F32, tag="psb")
                for b in range(NBP):
                    nc.tensor.matmul(psav[:], vblk(b)[:, l * D:(l + 1) * D],
                                     E[b][:, eh], start=(b == 0),
                                     stop=(b == NBP - 1))
                # denominator broadcast to all 128 rows, then fast recip
                psdn = ppsD.tile([128, 512], F32, tag="dn")
                nc.tensor.matmul(psdn[:], ones128[:], esum[:, eh],
                                 start=True, stop=True)
                rbf = prbf.tile([128, 512], F32, tag="rbf")
                nc.vector.reciprocal_approx_fast(out=rbf[:], in_=psdn[:])

                if qb == TB - 1:
                    # ---- CaM: bernoulli draw + rank-1 merge on t>=1792 ----
                    # sum over recent keys (packed >= 204): mask block 1
                    # (rows < 76 are start keys), then blocks 2..7
                    cs = slice(qo + 256, qo + 512)
                    e1m = ptmp.tile([128, 512], F16, tag="tmp")
                    nc.vector.tensor_scalar_mul(e1m[:, 0:256],
                                                E[1][:, cs],
                                                biases_sb[:, 1:2])
                    pssr = ppsD.tile([128, 256], F32, tag="dn")
                    nc.tensor.matmul(pssr[:], ones128[:], e1m[:, 0:256],
                                     start=True, stop=False)
                    for b in range(2, NBP):
                        nc.tensor.matmul(pssr[:], ones128[:],
                                         E[b][:, cs],
                                         start=False, stop=(b == NBP - 1))
                    # E row of evicted key 1229 = packed 204 (block 1, row 76)
                    erow = prow.tile([1, 256], F16, tag="row256")
                    nc.gpsimd.dma_start(erow[:], E[1][76:77, cs])
                    srec = prow.tile([1, 256], F32, tag="row256f")
                    nc.vector.tensor_sub(srec[:], pssr[0:1, :], erow[:])
                    # scalars at t = 2047 (col 255 of the 256-wide rows)
                    r_last = rbf[0:1, 511:512]
                    num = prow.tile([1, 1], F32, tag="sc")
                    nc.vector.tensor_mul(num[:], erow[0:1, 255:256], r_last)
                    mean = prow.tile([1, 1], F32, tag="sc")
                    nc.vector.tensor_mul(mean[:], srec[0:1, 255:256], r_last)
                    nc.vector.tensor_scalar_mul(mean[:], mean[:], 1.0 / 818.0)
                    nc.vector.tensor_scalar_add(mean[:], mean[:], 1e-6)
                    um = prow.tile([1, 1], F32, tag="sc")
                    nc.vector.tensor_mul(um[:], u2_sb[0:1, l:l + 1], mean[:])
                    bern = prow.tile([1, 1], F32, tag="sc")
                    nc.vector.tensor_tensor(bern[:], um[:], num[:],
                                            mybir.AluOpType.is_lt)
                    bs = prow.tile([1, 1], F32, tag="sc")
                    nc.vector.tensor_scalar_mul(bs[:], bern[:], 1.0 / RB)
                    coef = prow.tile([1, 256], F16, tag="row256")
                    nc.vector.tensor_scalar_mul(coef[:], srec[:], bs[:])
                    vrow = prow.tile([1, D], F16, tag="vrow")
                    nc.gpsimd.dma_start(vrow[:],
                                          vtp[1][76:77, l * D:(l + 1) * D])
                    pscr = ppsD.tile([128, 256], F32, tag="dn")
                    nc.tensor.matmul(pscr[:], vrow[:], coef[:],
                                     start=True, stop=True)
                # normalize columns by 1/denom
                nc.vector.tensor_mul(outT[l][:, qs], psav[:], rbf[:])
                if qb == TB - 1:
                    corr = ptmp.tile([128, 512], F16, tag="tmp")
                    nc.vector.tensor_mul(corr[:, 0:256], pscr[:],
                                         rbf[:, 256:512])
                    nc.vector.tensor_add(outT[l][:, 1792:2048],
                                         outT[l][:, 1792:2048],
                                         corr[:, 0:256])

            def o_proj(qb, fine=False):
                ost = postg.tile([128, DM], F16, tag="ostage")
                for i, ti in enumerate(range(qb * 4, qb * 4 + 4)):
                    for mb in range(TB):
                        pso = ppsB.tile([128, 512], F32, tag="psb")
                        for l in range(HL):
                            nc.tensor.matmul(
                                pso[:], outT[l][:, ti * 128:(ti + 1) * 128],
                                wo(l, mb), start=(l == 0), stop=(l == HL - 1))
                        dst = ost[:, mb * 512:(mb + 1) * 512]
                        if (i * TB + mb) % 2 == 0:
                            nc.scalar.copy(dst, pso[:])
                        else:
                            nc.vector.tensor_copy(dst, pso[:])
                        if fine and mb % 2 == 1:
                            nc.gpsimd.dma_start(
                                po[ti * 128:(ti + 1) * 128,
                                   (mb - 1) * 512:(mb + 1) * 512],
                                ost[:, (mb - 1) * 512:(mb + 1) * 512])
                    if not fine:
                        nc.sync.dma_start(po[ti * 128:(ti + 1) * 128, :],
                                          ost[:])
                    if ti != qb * 4 + 3:
                        ost = postg.tile([128, DM], F16, tag="ostage")

            pairs = [(2, 3), (0, 1)]
            combos = [(l, P) for P in pairs for l in range(HL)]
            pend = None
            for i, (l, P) in enumerate(combos):
                cur = scores_exp(l, P)
                if pend is not None:
                    pl, pP, pE, pes = pend
                    av_norm(pl, pP[0], pE, pes, 0)
                    av_norm(pl, pP[1], pE, pes, 512)
                if i == 3:
                    o_proj(pairs[0][0])
                    o_proj(pairs[0][1])
                pend = (l, P, *cur)
            pl, pP, pE, pes = pend
            av_norm(pl, pP[0], pE, pes, 0)
            av_norm(pl, pP[1], pE, pes, 512)
            o_proj(pairs[1][0])
            o_proj(pairs[1][1], fine=True)

    nc.compile()
    return nc


_NC_CACHE = None


def _get_nc():
    global _NC_CACHE
    if _NC_CACHE is None:
        _NC_CACHE = _build_nc()
    return _NC_CACHE


def _pack_w_lmajor(WT):
    # WT: [DM, JC] -> [128, HL*KC*128], head-major then kc
    out = []
    for l in range(HL):
        Wl = WT[:, l * 128:(l + 1) * 128]                  # [DM, 128]
        out.append(Wl.reshape(KC, 128, 128).transpose(1, 0, 2)
                   .reshape(128, KC * 128))
    return np.ascontiguousarray(np.concatenate(out, axis=1))


def make_in_maps(hidden_states, Wq, Wk, Wv, Wo):
    hs = np.asarray(hidden_states, np.float32).reshape(T, DM)
    hs = np.nan_to_num(hs, nan=0.0, posinf=1e4, neginf=-1e4)
    hsT = np.ascontiguousarray(hs.T.astype(np.float16))
    Wq = np.asarray(Wq, np.float32)
    Wk = np.asarray(Wk, np.float32)
    Wv = np.asarray(Wv, np.float32)
    Wo = np.asarray(Wo, np.float32)

    inv_freq = 1.0 / (10000.0 ** (np.arange(0, D, 2, dtype=np.float32) / D))
    freqs = np.arange(T, dtype=np.float32)[:, None] * inv_freq[None, :]
    emb = np.concatenate([freqs, freqs], axis=-1)          # [T, D]
    cosT = np.ascontiguousarray(np.cos(emb).T.astype(np.float16))
    sinT = np.sin(emb).T.astype(np.float32)
    sinTs = np.ascontiguousarray(
        np.concatenate([-sinT[:D // 2], sinT[D // 2:]], axis=0).astype(np.float16))

    u_full = np.asarray(UFULL, np.float32).reshape(1, H)

    bias_np = np.zeros((128, 2), np.float32)
    bias_np[127, 0] = -60000.0   # packed pad lane in block 7
    bias_np[76:, 1] = 1.0        # CaM recent-sum mask for block 1

    scale = 1.0 / np.sqrt(np.float32(D))
    in_maps = []
    for c in range(NCORES):
        js = slice(c * JC, (c + 1) * JC)
        in_maps.append({
            "hsT": hsT,
            "wqP": _pack_w_lmajor((Wq[js, :].T * scale).astype(np.float16)),
            "wkP": _pack_w_lmajor(Wk[js, :].T.astype(np.float16)),
            "wvP": np.ascontiguousarray(
                Wv[js, :].T.astype(np.float16).reshape(KC, 128, JC)
                .transpose(1, 0, 2).reshape(128, KC * JC)),
            "woP": np.ascontiguousarray(
                Wo[:, js].T.astype(np.float16).reshape(HL, 128, DM)
                .transpose(1, 0, 2).reshape(128, HL * DM)),
            "cosT": cosT,
            "sinTs": sinTs,
            "u2": np.ascontiguousarray(u_full[:, c * HL:(c + 1) * HL]),
            "biases": bias_np,
        })
    return in_maps


def kernel(hidden_states, Wq, Wk, Wv, Wo):
    nc = _get_nc()
    in_maps = make_in_maps(hidden_states, Wq, Wk, Wv, Wo)
    res = bass_utils.run_bass_kernel_spmd(nc, in_maps,
                                          core_ids=list(range(NCORES)))
    out = np.zeros((T, DM), np.float64)
    for c in range(NCORES):
        out += res.results[c]["po"].astype(np.float64)
    out = np.nan_to_num(out.astype(np.float32), nan=0.0, posinf=1e4,
                        neginf=-1e4)
    return out.reshape(1, T, DM)
